# revision 1
# baseline (speedup 1.0000x reference)
"""Trainium2 Bass kernel for nn_BidPrefix (segment_reduce).

Reference semantics, per row r (B=65536 rows, S=512 cols):
    cp[k]    = prod(x[r, 0:k])                  (exclusive prefix product)
    survival = cp[bid]
    rate     = cp[mp] - cp[mp+1], or EPS when mp == 0
returned as (survival [B,1] f32, rate_last [B,1] f32).

Design: masked products -- no cumprod materialisation, no dynamic
gather.  For each needed index k:
    cp[k] = reduce_mult_t( (t >= k) ? 1.0 : x[t] )
          = reduce_mult( max(x, (iota >= k)) )
The blend is ONE fused DVE scalar_tensor_tensor per (row-group, k):
    out = (iota cmp k[p]) max x    (cmp is_ge for k=bid,mp; is_gt for mp+1)
The {0,1} mask makes masked lanes contribute exactly 1.0 (multiplying by
1.0 is exact), so each product reproduces the reference's f32 prefix
product bit-for-bit, and k==0 (empty product == 1) needs no special
case.  All 3*G blends of a supertile land in one [128, 3G, 512] tile and
are reduced by a single 3D reduce_mult -> [128, 3G].

Supertile: [128, G*512] with row r = i*128*G + p*G + g (partition-major,
so each partition's DMA chunk is G*2KB contiguous).

Sharding: pure data parallel over the batch axis, B/8 = 8192 rows per
NeuronCore, same NEFF on all 8 cores (SPMD), outputs concatenated.
"""

import numpy as np

import concourse.bacc as bacc
import concourse.mybir as mybir
from concourse.tile import TileContext
from concourse.bass_utils import run_bass_kernel_spmd

f32 = mybir.dt.float32
i32 = mybir.dt.int32
Alu = mybir.AluOpType

N_CORES = 8
B, S = 65536, 512
ROWS = B // N_CORES          # 8192 rows per core
G = 8                        # 512-wide row-groups per supertile
SUPER = 128 * G              # 1024 rows per supertile
N_SUPER = ROWS // SUPER      # 8 supertiles per core
W = G * S
EPS = 1e-7


def build_bass():
    nc = bacc.Bacc()

    x = nc.dram_tensor("x", [ROWS, S], f32, kind="ExternalInput")
    bid_info = nc.dram_tensor("bid_info", [ROWS, 2], i32, kind="ExternalInput")
    surv_out = nc.dram_tensor("survival", [ROWS, 1], f32, kind="ExternalOutput")
    rate_out = nc.dram_tensor("rate_last", [ROWS, 1], f32, kind="ExternalOutput")

    x_v = x.rearrange("(i p g) s -> i p (g s)", p=128, g=G)
    bi_v = bid_info.rearrange("(i p g) c -> i p (g c)", p=128, g=G)
    so_v = surv_out.rearrange("(i p g) c -> i p (g c)", p=128, g=G)
    ro_v = rate_out.rearrange("(i p g) c -> i p (g c)", p=128, g=G)

    with TileContext(nc) as tc:
        with (
            tc.tile_pool(name="const", bufs=1) as cpool,
            tc.tile_pool(name="big", bufs=2) as bpool,
            tc.tile_pool(name="small", bufs=3) as spool,
        ):
            it512i = cpool.tile([128, 512], i32, tag="it512i")
            nc.gpsimd.iota(it512i[:], pattern=[[1, 512]], base=0,
                           channel_multiplier=0)
            it512 = cpool.tile([128, 512], f32, tag="it512")
            nc.vector.tensor_copy(out=it512[:], in_=it512i[:])

            for i in range(N_SUPER):
                xt = bpool.tile([128, W], f32, tag="xt")
                nc.sync.dma_start(out=xt[:], in_=x_v[i])
                bi = spool.tile([128, 2 * G], i32, tag="bi")
                nc.sync.dma_start(out=bi[:], in_=bi_v[i])

                bif = spool.tile([128, 2 * G], f32, tag="bif")
                nc.vector.tensor_copy(out=bif[:], in_=bi[:])
                bif3 = bif[:].rearrange("p (g c) -> p g c", c=2)
                mpf = bif3[:, :, 0:1]    # [128, G, 1] market price
                bidf = bif3[:, :, 1:2]   # [128, G, 1] bid

                # Tiny reads absorb the HWDGE per-queue semaphores before
                # the TensorScalarPtr-encoded STTs (that ISA encoding has
                # too few sync-wait slots to carry them itself).
                sink = spool.tile([128, 2], f32, tag="sink")
                nc.vector.tensor_copy(out=sink[:, 0:1], in_=xt[:, 0:1])

                # blends: BL[:, g*3+j, :]  j=0: k=bid, 1: k=mp, 2: k=mp+1
                BL = bpool.tile([128, 3 * G, S], f32, tag="BL")
                for g in range(G):
                    xg = xt[:, g * S:(g + 1) * S]
                    specs = [
                        (bidf[:, g, :], Alu.is_ge),
                        (mpf[:, g, :], Alu.is_ge),
                        (mpf[:, g, :], Alu.is_gt),
                    ]
                    for j, (kap, cmp) in enumerate(specs):
                        nc.vector.scalar_tensor_tensor(
                            out=BL[:, g * 3 + j, :], in0=it512[:],
                            scalar=kap, in1=xg, op0=cmp, op1=Alu.max)

                raw = spool.tile([128, 3 * G], f32, tag="raw")
                nc.vector.tensor_reduce(out=raw[:], in_=BL[:],
                                        axis=mybir.AxisListType.X, op=Alu.mult)
                raw3 = raw[:].rearrange("p (g j) -> p g j", j=3)
                svraw = raw3[:, :, 0]
                g1raw = raw3[:, :, 1]
                g2raw = raw3[:, :, 2]

                # rate = (g1-g2)*(1-(mp==0)) + EPS*(mp==0)  -- exact select
                m0m = spool.tile([128, G], f32, tag="m0m")
                nc.vector.tensor_scalar(out=m0m[:], in0=mpf, scalar1=0.0,
                                        scalar2=None, op0=Alu.is_equal)
                onem = spool.tile([128, G], f32, tag="onem")
                nc.vector.tensor_scalar(out=onem[:], in0=m0m[:], scalar1=-1.0,
                                        scalar2=1.0, op0=Alu.mult, op1=Alu.add)
                rate0 = spool.tile([128, G], f32, tag="rate0")
                nc.vector.tensor_sub(out=rate0[:], in0=g1raw, in1=g2raw)
                rate1 = spool.tile([128, G], f32, tag="rate1")
                nc.vector.tensor_mul(out=rate1[:], in0=rate0[:], in1=onem[:])
                rate_t = spool.tile([128, G], f32, tag="rate_t")
                nc.vector.scalar_tensor_tensor(
                    out=rate_t[:], in0=m0m[:], scalar=EPS, in1=rate1[:],
                    op0=Alu.mult, op1=Alu.add)

                nc.sync.dma_start(out=so_v[i], in_=svraw)
                nc.sync.dma_start(out=ro_v[i], in_=rate_t[:])
    nc.finalize()
    return nc


_NC_CACHE = None


def _get_nc():
    global _NC_CACHE
    if _NC_CACHE is None:
        _NC_CACHE = build_bass()
    return _NC_CACHE


def kernel(x, bid_info):
    x = np.ascontiguousarray(np.asarray(x, dtype=np.float32))
    bid_info = np.ascontiguousarray(np.asarray(bid_info, dtype=np.int32))
    assert x.shape == (B, S) and bid_info.shape == (B, 2)

    nc = _get_nc()
    in_maps = [
        {
            "x": x[c * ROWS:(c + 1) * ROWS],
            "bid_info": bid_info[c * ROWS:(c + 1) * ROWS],
        }
        for c in range(N_CORES)
    ]
    res = run_bass_kernel_spmd(nc, in_maps, core_ids=list(range(N_CORES)))
    survival = np.concatenate([r["survival"] for r in res.results], axis=0)
    rate_last = np.concatenate([r["rate_last"] for r in res.results], axis=0)
    return survival, rate_last



# revision 4
# speedup vs baseline: 2.6307x; 2.6307x over previous
"""Trainium2 Bass kernel for nn_BidPrefix (segment_reduce).

Reference semantics, per row r (B=65536 rows, S=512 cols):
    cp[k]    = prod(x[r, 0:k])                  (exclusive prefix product)
    survival = cp[bid]
    rate     = cp[mp] - cp[mp+1], or EPS when mp == 0
returned as (survival [B,1] f32, rate_last [B,1] f32).

Design (v2): exact fp32 inclusive cumprod via the DVE tensor_tensor_scan
(op0=mult, op1=bypass; one 512-long recurrence per row-group), then a
per-row 3-element extraction with ONE GPSIMD ap_gather per supertile:
    survival = cpi[bid-1]   (bid==0 -> 1, fixed up)
    g1       = cpi[mp-1]    (mp==0 handled by the EPS fixup)
    g2       = cpi[mp]
    rate     = mp==0 ? EPS : g1 - g2
ap_gather applies, for each 16-partition GPSIMD core, the index list
stored across its 16 partitions (slot s of partition p = flat position
q = s*16 + p%16) to ALL 16 channels; row p's own values therefore land
at out[p, s*16 + p%16] and are pulled out with a static one-hot mask
(mult + segmented reduce).  Indices are pre-offset by g*512 so a single
gather covers the whole [128, G*512] supertile.

Engine budget per core (modeled): DMA-in 8x5.8us (the memory roofline),
DVE 64 scans ~36us + small ops, Pool 8 gathers ~45us, Act idle.  All
three sit just under the DMA floor, so the kernel is memory-bound.

Row mapping r = p*64 + i*8 + g keeps every DMA contiguous per
partition: x in 16KB chunks, bid_info 512B, outputs 256B.

Sharding: pure data parallel over the batch axis, B/8 = 8192 rows per
NeuronCore, same NEFF on all 8 cores (SPMD), outputs concatenated.
"""

import numpy as np

import concourse.bacc as bacc
import concourse.mybir as mybir
from concourse.tile import TileContext
from concourse.bass_utils import run_bass_kernel_spmd

f32 = mybir.dt.float32
i32 = mybir.dt.int32
i16 = mybir.dt.int16
Alu = mybir.AluOpType
Ax = mybir.AxisListType

N_CORES = 8
B, S = 65536, 512
ROWS = B // N_CORES          # 8192 rows per core
G = 8                        # 512-wide row-groups per supertile
NI = 8                       # supertiles per core
K = NI * G                   # 64 rows per partition
W = G * S                    # 4096
NIDX = 3 * G * 16            # gather indices per gpsimd core (24 slots x 16)
EPS = 1e-7


def build_bass():
    nc = bacc.Bacc()

    x = nc.dram_tensor("x", [ROWS, S], f32, kind="ExternalInput")
    bid_info = nc.dram_tensor("bid_info", [ROWS, 2], i32, kind="ExternalInput")
    surv_out = nc.dram_tensor("survival", [ROWS, 1], f32, kind="ExternalOutput")
    rate_out = nc.dram_tensor("rate_last", [ROWS, 1], f32, kind="ExternalOutput")

    # row r = p*64 + i*8 + g  ->  all DMAs contiguous per partition
    x_v = x.rearrange("(p i g) s -> i p (g s)", p=128, i=NI, g=G)
    bi_v = bid_info.rearrange("(p k) c -> p (k c)", p=128)   # [128, 128]
    so_v = surv_out.rearrange("(p k) c -> p (k c)", p=128)   # [128, 64]
    ro_v = rate_out.rearrange("(p k) c -> p (k c)", p=128)

    with TileContext(nc) as tc:
        with (
            tc.tile_pool(name="const", bufs=1) as cpool,
            tc.tile_pool(name="big", bufs=2) as bpool,
            tc.tile_pool(name="small", bufs=3) as spool,
        ):
            # ---- static constants (gpsimd iotas first: one library load) --
            rq = cpool.tile([128, 24 * 16], i32, tag="rq")     # q%16 per slot
            nc.gpsimd.iota(rq[:], pattern=[[0, 24], [1, 16]], base=0,
                           channel_multiplier=0)
            pp = cpool.tile([128, 1], i32, tag="pp")           # partition idx
            nc.gpsimd.iota(pp[:], pattern=[[1, 1]], base=0,
                           channel_multiplier=1)
            offs32 = cpool.tile([128, 3 * K], i32, tag="offs32")  # g*512
            nc.gpsimd.iota(offs32[:], pattern=[[0, NI], [512, G], [0, 3]],
                           base=0, channel_multiplier=0)

            pm16 = cpool.tile([128, 1], i32, tag="pm16")
            nc.vector.tensor_scalar(out=pm16[:], in0=pp[:], scalar1=15,
                                    scalar2=None, op0=Alu.bitwise_and)
            pm16f = cpool.tile([128, 1], f32, tag="pm16f")
            nc.vector.tensor_copy(out=pm16f[:], in_=pm16[:])
            rqf = cpool.tile([128, 24 * 16], f32, tag="rqf")
            nc.vector.tensor_copy(out=rqf[:], in_=rq[:])
            # one-hot diag mask M[p, s*16+r] = (r == p%16)
            m384 = cpool.tile([128, 24 * 16], f32, tag="m384")
            nc.vector.tensor_scalar(out=m384[:], in0=rqf[:], scalar1=pm16f[:],
                                    scalar2=None, op0=Alu.is_equal)
            offsf = cpool.tile([128, 3 * K], f32, tag="offsf")
            nc.vector.tensor_copy(out=offsf[:], in_=offs32[:])

            # ---- bid_info load + index prep (f32 domain, cast to i16) ----
            bi_all = cpool.tile([128, 2 * K], i32, tag="bi_all")
            nc.sync.dma_start(out=bi_all[:], in_=bi_v)
            bif = cpool.tile([128, 2 * K], f32, tag="bif")
            nc.vector.tensor_copy(out=bif[:], in_=bi_all[:])
            bif3 = bif[:].rearrange("p (k c) -> p k c", c=2)
            mpf = bif3[:, :, 0]     # [128, 64] market price (strided)
            bidf = bif3[:, :, 1]    # [128, 64] bid

            idxf = cpool.tile([128, 3 * K], f32, tag="idxf")
            idx3 = idxf[:].rearrange("p (k j) -> p k j", j=3)
            # j=0: bid-1, j=1: mp-1, j=2: mp   (clamped at 0; fixups later)
            nc.vector.tensor_scalar(out=idx3[:, :, 0], in0=bidf, scalar1=-1.0,
                                    scalar2=0.0, op0=Alu.add, op1=Alu.max)
            nc.vector.tensor_scalar(out=idx3[:, :, 1], in0=mpf, scalar1=-1.0,
                                    scalar2=0.0, op0=Alu.add, op1=Alu.max)
            nc.vector.tensor_copy(out=idx3[:, :, 2], in_=mpf)
            nc.vector.tensor_tensor(out=idxf[:], in0=idxf[:], in1=offsf[:],
                                    op=Alu.add)
            idx16 = cpool.tile([128, 3 * K], i16, tag="idx16")
            nc.vector.tensor_copy(out=idx16[:], in_=idxf[:])

            vals = cpool.tile([128, 3 * K], f32, tag="vals")  # [p,(i,g),j]

            # ---- main loop over supertiles --------------------------------
            for i in range(NI):
                xt = bpool.tile([128, W], f32, tag="xt")
                nc.sync.dma_start(out=xt[:], in_=x_v[i])

                # Tiny read absorbs the HWDGE queue semaphore before the
                # TensorScalarPtr-encoded scans (that ISA encoding has too
                # few sync-wait slots to carry it itself).
                sink = spool.tile([128, 2], f32, tag="sink")
                nc.vector.tensor_copy(out=sink[:, 0:1], in_=xt[:, 0:1])

                cp = bpool.tile([128, W], f32, tag="cp")
                for g in range(G):
                    sl = slice(g * S, (g + 1) * S)
                    nc.vector.tensor_tensor_scan(
                        out=cp[:, sl], data0=xt[:, sl], data1=xt[:, sl],
                        initial=1.0, op0=Alu.mult, op1=Alu.bypass)

                gat = spool.tile([128, 24 * 16], f32, tag="gat")
                nc.gpsimd.ap_gather(
                    out_ap=gat[:], in_ap=cp[:],
                    idxs_ap=idx16[:, i * 24:(i + 1) * 24],
                    channels=128, num_elems=W, d=1, num_idxs=NIDX)

                msk = spool.tile([128, 24 * 16], f32, tag="msk")
                nc.vector.tensor_tensor(out=msk[:], in0=gat[:], in1=m384[:],
                                        op=Alu.mult)
                m3 = msk[:].rearrange("p (s r) -> p s r", r=16)
                nc.vector.tensor_reduce(out=vals[:, i * 24:(i + 1) * 24],
                                        in_=m3, axis=Ax.X, op=Alu.add)

            # ---- fixups + store ------------------------------------------
            v3 = vals[:].rearrange("p (k j) -> p k j", j=3)
            sv_raw = v3[:, :, 0]    # cpi[bid-1] (garbage when bid==0)
            g1v = v3[:, :, 1]       # cpi[mp-1]  (garbage when mp==0)
            g2v = v3[:, :, 2]       # cpi[mp]

            mb = cpool.tile([128, K], f32, tag="mb")
            nc.vector.tensor_scalar(out=mb[:], in0=bidf, scalar1=0.0,
                                    scalar2=None, op0=Alu.is_equal)
            mm = cpool.tile([128, K], f32, tag="mm")
            nc.vector.tensor_scalar(out=mm[:], in0=mpf, scalar1=0.0,
                                    scalar2=None, op0=Alu.is_equal)

            # survival = sv_raw + mb*(1 - sv_raw)
            t1 = cpool.tile([128, K], f32, tag="t1")
            nc.vector.tensor_scalar(out=t1[:], in0=sv_raw, scalar1=-1.0,
                                    scalar2=1.0, op0=Alu.mult, op1=Alu.add)
            t2 = cpool.tile([128, K], f32, tag="t2")
            nc.vector.tensor_tensor(out=t2[:], in0=mb[:], in1=t1[:],
                                    op=Alu.mult)
            surv = cpool.tile([128, K], f32, tag="surv")
            nc.vector.tensor_tensor(out=surv[:], in0=t2[:], in1=sv_raw,
                                    op=Alu.add)

            # rate = r0 + mm*(EPS - r0),  r0 = g1 - g2
            r0 = cpool.tile([128, K], f32, tag="r0")
            nc.vector.tensor_tensor(out=r0[:], in0=g1v, in1=g2v,
                                    op=Alu.subtract)
            t3 = cpool.tile([128, K], f32, tag="t3")
            nc.vector.tensor_scalar(out=t3[:], in0=r0[:], scalar1=-1.0,
                                    scalar2=EPS, op0=Alu.mult, op1=Alu.add)
            t4 = cpool.tile([128, K], f32, tag="t4")
            nc.vector.tensor_tensor(out=t4[:], in0=mm[:], in1=t3[:],
                                    op=Alu.mult)
            rate = cpool.tile([128, K], f32, tag="rate")
            nc.vector.tensor_tensor(out=rate[:], in0=t4[:], in1=r0[:],
                                    op=Alu.add)

            nc.sync.dma_start(out=so_v, in_=surv[:])
            nc.sync.dma_start(out=ro_v, in_=rate[:])
    nc.finalize()
    return nc


_NC_CACHE = None


def _get_nc():
    global _NC_CACHE
    if _NC_CACHE is None:
        _NC_CACHE = build_bass()
    return _NC_CACHE


def kernel(x, bid_info):
    x = np.ascontiguousarray(np.asarray(x, dtype=np.float32))
    bid_info = np.ascontiguousarray(np.asarray(bid_info, dtype=np.int32))
    assert x.shape == (B, S) and bid_info.shape == (B, 2)

    nc = _get_nc()
    in_maps = [
        {
            "x": x[c * ROWS:(c + 1) * ROWS],
            "bid_info": bid_info[c * ROWS:(c + 1) * ROWS],
        }
        for c in range(N_CORES)
    ]
    res = run_bass_kernel_spmd(nc, in_maps, core_ids=list(range(N_CORES)))
    survival = np.concatenate([r["survival"] for r in res.results], axis=0)
    rate_last = np.concatenate([r["rate_last"] for r in res.results], axis=0)
    return survival, rate_last


# revision 5
# speedup vs baseline: 3.3213x; 1.2625x over previous
"""Trainium2 Bass kernel for nn_BidPrefix (segment_reduce).

Reference semantics, per row r (B=65536 rows, S=512 cols):
    cp[k]    = prod(x[r, 0:k])                  (exclusive prefix product)
    survival = cp[bid]
    rate     = cp[mp] - cp[mp+1], or EPS when mp == 0
returned as (survival [B,1] f32, rate_last [B,1] f32).

Design (v2): exact fp32 inclusive cumprod via the DVE tensor_tensor_scan
(op0=mult, op1=bypass; one 512-long recurrence per row-group), then a
per-row 3-element extraction with ONE GPSIMD ap_gather per supertile:
    survival = cpi[bid-1]   (bid==0 -> 1, fixed up)
    g1       = cpi[mp-1]    (mp==0 handled by the EPS fixup)
    g2       = cpi[mp]
    rate     = mp==0 ? EPS : g1 - g2
ap_gather applies, for each 16-partition GPSIMD core, the index list
stored across its 16 partitions (slot s of partition p = flat position
q = s*16 + p%16) to ALL 16 channels; row p's own values therefore land
at out[p, s*16 + p%16] and are pulled out with a static one-hot mask
(mult + segmented reduce).  Indices are pre-offset by g*512 so a single
gather covers the whole [128, G*512] supertile.

Engine budget per core (modeled): DMA-in 8x5.8us (the memory roofline),
DVE 64 scans ~36us + small ops, Pool 8 gathers ~45us, Act idle.  All
three sit just under the DMA floor, so the kernel is memory-bound.

Row mapping r = p*64 + i*8 + g keeps every DMA contiguous per
partition: x in 16KB chunks, bid_info 512B, outputs 256B.

Sharding: pure data parallel over the batch axis, B/8 = 8192 rows per
NeuronCore, same NEFF on all 8 cores (SPMD), outputs concatenated.
"""

import numpy as np

import concourse.bacc as bacc
import concourse.mybir as mybir
from concourse.tile import TileContext
from concourse.bass_utils import run_bass_kernel_spmd

f32 = mybir.dt.float32
i32 = mybir.dt.int32
i16 = mybir.dt.int16
Alu = mybir.AluOpType
Ax = mybir.AxisListType

N_CORES = 8
B, S = 65536, 512
ROWS = B // N_CORES          # 8192 rows per core
G = 8                        # 512-wide row-groups per supertile
NI = 8                       # supertiles per core
K = NI * G                   # 64 rows per partition
W = G * S                    # 4096
NIDX = 3 * G * 16            # gather indices per gpsimd core (24 slots x 16)
EPS = 1e-7


def build_bass():
    nc = bacc.Bacc()

    x = nc.dram_tensor("x", [ROWS, S], f32, kind="ExternalInput")
    bid_info = nc.dram_tensor("bid_info", [ROWS, 2], i32, kind="ExternalInput")
    surv_out = nc.dram_tensor("survival", [ROWS, 1], f32, kind="ExternalOutput")
    rate_out = nc.dram_tensor("rate_last", [ROWS, 1], f32, kind="ExternalOutput")

    # row r = p*64 + i*8 + g  ->  all DMAs contiguous per partition
    x_v = x.rearrange("(p i g) s -> i p (g s)", p=128, i=NI, g=G)
    bi_v = bid_info.rearrange("(p k) c -> p (k c)", p=128)   # [128, 128]
    so_v = surv_out.rearrange("(p k) c -> p (k c)", p=128)   # [128, 64]
    ro_v = rate_out.rearrange("(p k) c -> p (k c)", p=128)

    with TileContext(nc) as tc:
        with (
            tc.tile_pool(name="const", bufs=1) as cpool,
            tc.tile_pool(name="big", bufs=2) as bpool,
            tc.tile_pool(name="small", bufs=3) as spool,
        ):
            # ---- static constants (gpsimd iotas first: one library load) --
            rq = cpool.tile([128, 24 * 16], i32, tag="rq")     # q%16 per slot
            nc.gpsimd.iota(rq[:], pattern=[[0, 24], [1, 16]], base=0,
                           channel_multiplier=0)
            pp = cpool.tile([128, 1], i32, tag="pp")           # partition idx
            nc.gpsimd.iota(pp[:], pattern=[[1, 1]], base=0,
                           channel_multiplier=1)
            offs32 = cpool.tile([128, 3 * K], i32, tag="offs32")  # g*512
            nc.gpsimd.iota(offs32[:], pattern=[[0, NI], [512, G], [0, 3]],
                           base=0, channel_multiplier=0)

            pm16 = cpool.tile([128, 1], i32, tag="pm16")
            nc.vector.tensor_scalar(out=pm16[:], in0=pp[:], scalar1=15,
                                    scalar2=None, op0=Alu.bitwise_and)
            pm16f = cpool.tile([128, 1], f32, tag="pm16f")
            nc.vector.tensor_copy(out=pm16f[:], in_=pm16[:])
            rqf = cpool.tile([128, 24 * 16], f32, tag="rqf")
            nc.vector.tensor_copy(out=rqf[:], in_=rq[:])
            # one-hot diag mask M[p, s*16+r] = (r == p%16)
            m384 = cpool.tile([128, 24 * 16], f32, tag="m384")
            nc.vector.tensor_scalar(out=m384[:], in0=rqf[:], scalar1=pm16f[:],
                                    scalar2=None, op0=Alu.is_equal)
            offsf = cpool.tile([128, 3 * K], f32, tag="offsf")
            nc.vector.tensor_copy(out=offsf[:], in_=offs32[:])

            # ---- bid_info load + index prep (f32 domain, cast to i16) ----
            bi_all = cpool.tile([128, 2 * K], i32, tag="bi_all")
            nc.sync.dma_start(out=bi_all[:], in_=bi_v)
            bif = cpool.tile([128, 2 * K], f32, tag="bif")
            nc.vector.tensor_copy(out=bif[:], in_=bi_all[:])
            bif3 = bif[:].rearrange("p (k c) -> p k c", c=2)
            mpf = bif3[:, :, 0]     # [128, 64] market price (strided)
            bidf = bif3[:, :, 1]    # [128, 64] bid

            idxf = cpool.tile([128, 3 * K], f32, tag="idxf")
            idx3 = idxf[:].rearrange("p (k j) -> p k j", j=3)
            # j=0: bid-1, j=1: mp-1, j=2: mp   (clamped at 0; fixups later)
            nc.vector.tensor_scalar(out=idx3[:, :, 0], in0=bidf, scalar1=-1.0,
                                    scalar2=0.0, op0=Alu.add, op1=Alu.max)
            nc.vector.tensor_scalar(out=idx3[:, :, 1], in0=mpf, scalar1=-1.0,
                                    scalar2=0.0, op0=Alu.add, op1=Alu.max)
            nc.vector.tensor_copy(out=idx3[:, :, 2], in_=mpf)
            nc.vector.tensor_tensor(out=idxf[:], in0=idxf[:], in1=offsf[:],
                                    op=Alu.add)
            idx16 = cpool.tile([128, 3 * K], i16, tag="idx16")
            nc.vector.tensor_copy(out=idx16[:], in_=idxf[:])

            vals = cpool.tile([128, 3 * K], f32, tag="vals")  # [p,(i,g),j]

            # ---- main loop over supertiles --------------------------------
            # Software-pipelined: extract for supertile i-1 is issued after
            # the scans of supertile i, so the in-order DVE never stalls on
            # the Pool gather it consumes (that gather ran during these
            # scans).
            def extract(i, gat):
                msk = spool.tile([128, 24 * 16], f32, tag="msk")
                nc.vector.tensor_tensor(out=msk[:], in0=gat[:], in1=m384[:],
                                        op=Alu.mult)
                m3 = msk[:].rearrange("p (s r) -> p s r", r=16)
                nc.vector.tensor_reduce(out=vals[:, i * 24:(i + 1) * 24],
                                        in_=m3, axis=Ax.X, op=Alu.add)

            prev = None  # (i, gat) awaiting extraction
            for i in range(NI):
                xt = bpool.tile([128, W], f32, tag="xt")
                nc.sync.dma_start(out=xt[:], in_=x_v[i])

                # Tiny read absorbs the HWDGE queue semaphore before the
                # TensorScalarPtr-encoded scans (that ISA encoding has too
                # few sync-wait slots to carry it itself).
                sink = spool.tile([128, 2], f32, tag="sink")
                nc.vector.tensor_copy(out=sink[:, 0:1], in_=xt[:, 0:1])

                cp = bpool.tile([128, W], f32, tag="cp")
                for g in range(G):
                    sl = slice(g * S, (g + 1) * S)
                    nc.vector.tensor_tensor_scan(
                        out=cp[:, sl], data0=xt[:, sl], data1=xt[:, sl],
                        initial=1.0, op0=Alu.mult, op1=Alu.bypass)

                gat = spool.tile([128, 24 * 16], f32, tag="gat")
                nc.gpsimd.ap_gather(
                    out_ap=gat[:], in_ap=cp[:],
                    idxs_ap=idx16[:, i * 24:(i + 1) * 24],
                    channels=128, num_elems=W, d=1, num_idxs=NIDX)

                if prev is not None:
                    extract(*prev)
                prev = (i, gat)
            extract(*prev)

            # ---- fixups + store ------------------------------------------
            v3 = vals[:].rearrange("p (k j) -> p k j", j=3)
            sv_raw = v3[:, :, 0]    # cpi[bid-1] (garbage when bid==0)
            g1v = v3[:, :, 1]       # cpi[mp-1]  (garbage when mp==0)
            g2v = v3[:, :, 2]       # cpi[mp]

            mb = cpool.tile([128, K], f32, tag="mb")
            nc.vector.tensor_scalar(out=mb[:], in0=bidf, scalar1=0.0,
                                    scalar2=None, op0=Alu.is_equal)
            mm = cpool.tile([128, K], f32, tag="mm")
            nc.vector.tensor_scalar(out=mm[:], in0=mpf, scalar1=0.0,
                                    scalar2=None, op0=Alu.is_equal)

            # survival = sv_raw + mb*(1 - sv_raw)
            t1 = cpool.tile([128, K], f32, tag="t1")
            nc.vector.tensor_scalar(out=t1[:], in0=sv_raw, scalar1=-1.0,
                                    scalar2=1.0, op0=Alu.mult, op1=Alu.add)
            t2 = cpool.tile([128, K], f32, tag="t2")
            nc.vector.tensor_tensor(out=t2[:], in0=mb[:], in1=t1[:],
                                    op=Alu.mult)
            surv = cpool.tile([128, K], f32, tag="surv")
            nc.vector.tensor_tensor(out=surv[:], in0=t2[:], in1=sv_raw,
                                    op=Alu.add)

            # rate = r0 + mm*(EPS - r0),  r0 = g1 - g2
            r0 = cpool.tile([128, K], f32, tag="r0")
            nc.vector.tensor_tensor(out=r0[:], in0=g1v, in1=g2v,
                                    op=Alu.subtract)
            t3 = cpool.tile([128, K], f32, tag="t3")
            nc.vector.tensor_scalar(out=t3[:], in0=r0[:], scalar1=-1.0,
                                    scalar2=EPS, op0=Alu.mult, op1=Alu.add)
            t4 = cpool.tile([128, K], f32, tag="t4")
            nc.vector.tensor_tensor(out=t4[:], in0=mm[:], in1=t3[:],
                                    op=Alu.mult)
            rate = cpool.tile([128, K], f32, tag="rate")
            nc.vector.tensor_tensor(out=rate[:], in0=t4[:], in1=r0[:],
                                    op=Alu.add)

            nc.sync.dma_start(out=so_v, in_=surv[:])
            nc.sync.dma_start(out=ro_v, in_=rate[:])
    nc.finalize()
    return nc


_NC_CACHE = None


def _get_nc():
    global _NC_CACHE
    if _NC_CACHE is None:
        _NC_CACHE = build_bass()
    return _NC_CACHE


def kernel(x, bid_info):
    x = np.ascontiguousarray(np.asarray(x, dtype=np.float32))
    bid_info = np.ascontiguousarray(np.asarray(bid_info, dtype=np.int32))
    assert x.shape == (B, S) and bid_info.shape == (B, 2)

    nc = _get_nc()
    in_maps = [
        {
            "x": x[c * ROWS:(c + 1) * ROWS],
            "bid_info": bid_info[c * ROWS:(c + 1) * ROWS],
        }
        for c in range(N_CORES)
    ]
    res = run_bass_kernel_spmd(nc, in_maps, core_ids=list(range(N_CORES)))
    survival = np.concatenate([r["survival"] for r in res.results], axis=0)
    rate_last = np.concatenate([r["rate_last"] for r in res.results], axis=0)
    return survival, rate_last


# revision 7
# speedup vs baseline: 3.4940x; 1.0520x over previous
"""Trainium2 Bass kernel for nn_BidPrefix (segment_reduce).

Reference semantics, per row r (B=65536 rows, S=512 cols):
    cp[k]    = prod(x[r, 0:k])                  (exclusive prefix product)
    survival = cp[bid]
    rate     = cp[mp] - cp[mp+1], or EPS when mp == 0
returned as (survival [B,1] f32, rate_last [B,1] f32).

Design (v4): exact fp32 inclusive cumprod via DVE tensor_tensor_scan
(op0=mult, op1=bypass; one 512-long recurrence per row-group, written
in place over the x tile), then a per-row 3-element extraction with one
GPSIMD ap_gather per chunk:
    survival = cpi[bid-1]   (bid==0 -> 1, fixed up)
    g1       = cpi[mp-1]    (mp==0 handled by the EPS fixup)
    g2       = cpi[mp]
    rate     = mp==0 ? EPS : g1 - g2
ap_gather applies, for each 16-partition GPSIMD core, the index list
stored across its 16 partitions (slot s of partition p = flat position
q = s*16 + p%16) to ALL 16 channels; row p's own values land at
out[p, s*16 + p%16] and are pulled out with a static one-hot mask
(mult + segmented reduce).  Indices are pre-offset by glocal*512 so a
single gather covers a whole multi-group chunk.

Schedule: the three ~47us engine streams (DMA-in, DVE scans+extracts,
Pool gathers) are software-pipelined: extract for chunk c-1 is issued
after the scans of chunk c so the in-order DVE never stalls on the
Pool gather it consumes.  Chunk sizes taper (4,8...8,2,2 groups) to
shorten pipeline fill and drain; index prep runs on the (otherwise
idle until the first gather) Pool engine; fixups are split so only the
last chunk's sliver trails the final gather.

Row mapping r = p*64 + k keeps every DMA contiguous per partition:
x chunks 4-16KB, bid_info 512B, outputs 256B.

Sharding: pure data parallel over the batch axis, B/8 = 8192 rows per
NeuronCore, same NEFF on all 8 cores (SPMD), outputs concatenated.
"""

import numpy as np

import concourse.bacc as bacc
import concourse.mybir as mybir
from concourse.tile import TileContext
from concourse.bass_utils import run_bass_kernel_spmd

f32 = mybir.dt.float32
i32 = mybir.dt.int32
i16 = mybir.dt.int16
Alu = mybir.AluOpType
Ax = mybir.AxisListType

N_CORES = 8
B, S = 65536, 512
ROWS = B // N_CORES          # 8192 rows per core
K = ROWS // 128              # 64 row-groups per partition
CHUNKS = [4, 8, 8, 8, 8, 8, 8, 8, 2, 2]   # groups per chunk, sum = 64
assert sum(CHUNKS) == K
EPS = 1e-7


def build_bass():
    nc = bacc.Bacc()

    x = nc.dram_tensor("x", [ROWS, S], f32, kind="ExternalInput")
    bid_info = nc.dram_tensor("bid_info", [ROWS, 2], i32, kind="ExternalInput")
    surv_out = nc.dram_tensor("survival", [ROWS, 1], f32, kind="ExternalOutput")
    rate_out = nc.dram_tensor("rate_last", [ROWS, 1], f32, kind="ExternalOutput")

    # row r = p*64 + k  ->  every DMA contiguous per partition
    x_v = x.rearrange("(p k) s -> p (k s)", p=128)           # [128, 64*512]
    bi_v = bid_info.rearrange("(p k) c -> p (k c)", p=128)   # [128, 128]
    so_v = surv_out.rearrange("(p k) c -> p (k c)", p=128)   # [128, 64]
    ro_v = rate_out.rearrange("(p k) c -> p (k c)", p=128)

    with TileContext(nc) as tc:
        with (
            tc.tile_pool(name="const", bufs=1) as cpool,
            tc.tile_pool(name="big8", bufs=4) as pool8,
            tc.tile_pool(name="big4", bufs=2) as pool4,
            tc.tile_pool(name="big2", bufs=2) as pool2,
            tc.tile_pool(name="small", bufs=3) as spool,
        ):
            pools = {8: pool8, 4: pool4, 2: pool2}

            # ---- first x chunk DMA goes out before anything else ----------
            g0 = CHUNKS[0]
            xt0 = pools[g0].tile([128, g0 * S], f32, tag=f"xt{g0}")
            nc.sync.dma_start(out=xt0[:], in_=x_v[:, 0:g0 * S])
            bi_all = cpool.tile([128, 2 * K], i32, tag="bi_all")
            nc.sync.dma_start(out=bi_all[:], in_=bi_v)

            # ---- static constants + index prep, all on Pool (idle until
            # the first gather) so the DVE can start scanning immediately --
            rq = cpool.tile([128, 24 * 16], i32, tag="rq")     # q%16 per slot
            nc.gpsimd.iota(rq[:], pattern=[[0, 24], [1, 16]], base=0,
                           channel_multiplier=0)
            pp = cpool.tile([128, 1], i32, tag="pp")           # partition idx
            nc.gpsimd.iota(pp[:], pattern=[[1, 1]], base=0,
                           channel_multiplier=1)
            offs32 = cpool.tile([128, 3 * K], i32, tag="offs32")  # glocal*512
            gs = 0
            for gc in CHUNKS:
                nc.gpsimd.iota(offs32[:, 3 * gs:3 * (gs + gc)],
                               pattern=[[512, gc], [0, 3]], base=0,
                               channel_multiplier=0)
                gs += gc

            # Pool handles the casts (proven gpsimd ops); the
            # TensorScalarPtr-encoded arithmetic must stay on DVE (walrus
            # rejects that encoding on Pool).
            pm16 = cpool.tile([128, 1], i32, tag="pm16")
            nc.vector.tensor_scalar(out=pm16[:], in0=pp[:], scalar1=15,
                                    scalar2=None, op0=Alu.bitwise_and)
            pm16f = cpool.tile([128, 1], f32, tag="pm16f")
            nc.gpsimd.tensor_copy(out=pm16f[:], in_=pm16[:])
            rqf = cpool.tile([128, 24 * 16], f32, tag="rqf")
            nc.gpsimd.tensor_copy(out=rqf[:], in_=rq[:])
            # one-hot diag mask M[p, s*16+r] = (r == p%16), periodic in s
            m384 = cpool.tile([128, 24 * 16], f32, tag="m384")
            nc.vector.tensor_scalar(out=m384[:], in0=rqf[:], scalar1=pm16f[:],
                                    scalar2=None, op0=Alu.is_equal)
            offsf = cpool.tile([128, 3 * K], f32, tag="offsf")
            nc.gpsimd.tensor_copy(out=offsf[:], in_=offs32[:])

            bif = cpool.tile([128, 2 * K], f32, tag="bif")
            nc.gpsimd.tensor_copy(out=bif[:], in_=bi_all[:])
            bif3 = bif[:].rearrange("p (k c) -> p k c", c=2)
            mpf = bif3[:, :, 0]     # [128, 64] market price (strided)
            bidf = bif3[:, :, 1]    # [128, 64] bid

            idxf = cpool.tile([128, 3 * K], f32, tag="idxf")
            idx3 = idxf[:].rearrange("p (k j) -> p k j", j=3)
            # j=0: bid-1, j=1: mp-1, j=2: mp   (clamped at 0; fixups later)
            nc.vector.tensor_scalar(out=idx3[:, :, 0], in0=bidf, scalar1=-1.0,
                                    scalar2=0.0, op0=Alu.add, op1=Alu.max)
            nc.vector.tensor_scalar(out=idx3[:, :, 1], in0=mpf, scalar1=-1.0,
                                    scalar2=0.0, op0=Alu.add, op1=Alu.max)
            nc.gpsimd.tensor_copy(out=idx3[:, :, 2], in_=mpf)
            nc.vector.tensor_tensor(out=idxf[:], in0=idxf[:], in1=offsf[:],
                                    op=Alu.add)
            idx16 = cpool.tile([128, 3 * K], i16, tag="idx16")
            nc.gpsimd.tensor_copy(out=idx16[:], in_=idxf[:])

            # masks for the bid==0 / mp==0 fixups (read only at the end)
            mb = cpool.tile([128, K], f32, tag="mb")
            nc.vector.tensor_scalar(out=mb[:], in0=bidf, scalar1=0.0,
                                    scalar2=None, op0=Alu.is_equal)
            mm = cpool.tile([128, K], f32, tag="mm")
            nc.vector.tensor_scalar(out=mm[:], in0=mpf, scalar1=0.0,
                                    scalar2=None, op0=Alu.is_equal)

            vals = cpool.tile([128, 3 * K], f32, tag="vals")  # [p, k, j]
            surv = cpool.tile([128, K], f32, tag="surv")
            rate = cpool.tile([128, K], f32, tag="rate")

            v3 = vals[:].rearrange("p (k j) -> p k j", j=3)

            def extract(gs, gc, gat):
                msk = spool.tile([128, 48 * gc], f32, tag=f"msk{gc}")
                nc.vector.tensor_tensor(out=msk[:], in0=gat[:],
                                        in1=m384[:, 0:48 * gc], op=Alu.mult)
                m3 = msk[:].rearrange("p (s r) -> p s r", r=16)
                nc.vector.tensor_reduce(out=vals[:, 3 * gs:3 * (gs + gc)],
                                        in_=m3, axis=Ax.X, op=Alu.add)

            def fixups(lo, hi):
                sl = slice(lo, hi)
                sv_raw = v3[:, sl, 0]   # cpi[bid-1] (garbage when bid==0)
                g1v = v3[:, sl, 1]      # cpi[mp-1]  (garbage when mp==0)
                g2v = v3[:, sl, 2]      # cpi[mp]
                n = hi - lo
                # survival = sv_raw + mb*(1 - sv_raw)
                t1 = spool.tile([128, n], f32, tag=f"t1{n}")
                nc.vector.tensor_scalar(out=t1[:], in0=sv_raw, scalar1=-1.0,
                                        scalar2=1.0, op0=Alu.mult, op1=Alu.add)
                t2 = spool.tile([128, n], f32, tag=f"t2{n}")
                nc.vector.tensor_tensor(out=t2[:], in0=mb[:, sl], in1=t1[:],
                                        op=Alu.mult)
                nc.vector.tensor_tensor(out=surv[:, sl], in0=t2[:], in1=sv_raw,
                                        op=Alu.add)
                # rate = r0 + mm*(EPS - r0),  r0 = g1 - g2
                r0 = spool.tile([128, n], f32, tag=f"r0{n}")
                nc.vector.tensor_tensor(out=r0[:], in0=g1v, in1=g2v,
                                        op=Alu.subtract)
                t3 = spool.tile([128, n], f32, tag=f"t3{n}")
                nc.vector.tensor_scalar(out=t3[:], in0=r0[:], scalar1=-1.0,
                                        scalar2=EPS, op0=Alu.mult, op1=Alu.add)
                t4 = spool.tile([128, n], f32, tag=f"t4{n}")
                nc.vector.tensor_tensor(out=t4[:], in0=mm[:, sl], in1=t3[:],
                                        op=Alu.mult)
                nc.vector.tensor_tensor(out=rate[:, sl], in0=t4[:], in1=r0[:],
                                        op=Alu.add)

            # ---- main loop over chunks -----------------------------------
            prev = None  # (gs, gc, gat) awaiting extraction
            gs = 0
            n_chunks = len(CHUNKS)
            for ci, gc in enumerate(CHUNKS):
                if ci == 0:
                    xt = xt0
                else:
                    xt = pools[gc].tile([128, gc * S], f32, tag=f"xt{gc}")
                    nc.sync.dma_start(out=xt[:],
                                      in_=x_v[:, gs * S:(gs + gc) * S])

                # Tiny read absorbs the HWDGE queue semaphore before the
                # TensorScalarPtr-encoded scans (that ISA encoding has too
                # few sync-wait slots to carry it itself).
                sink = spool.tile([128, 2], f32, tag="sink")
                nc.vector.tensor_copy(out=sink[:, 0:1], in_=xt[:, 0:1])

                for g in range(gc):
                    sl = slice(g * S, (g + 1) * S)
                    nc.vector.tensor_tensor_scan(
                        out=xt[:, sl], data0=xt[:, sl], data1=xt[:, sl],
                        initial=1.0, op0=Alu.mult, op1=Alu.bypass)

                gat = spool.tile([128, 48 * gc], f32, tag=f"gat{gc}")
                nc.gpsimd.ap_gather(
                    out_ap=gat[:], in_ap=xt[:],
                    idxs_ap=idx16[:, 3 * gs:3 * (gs + gc)],
                    channels=128, num_elems=gc * S, d=1, num_idxs=48 * gc)

                if prev is not None:
                    extract(*prev)
                prev = (gs, gc, gat)
                if ci == n_chunks - 1:
                    # head fixups run on DVE while the last gather runs
                    fixups(0, gs)
                gs += gc

            extract(*prev)
            fixups(K - CHUNKS[-1], K)

            nc.sync.dma_start(out=so_v, in_=surv[:])
            nc.sync.dma_start(out=ro_v, in_=rate[:])
    nc.finalize()
    return nc


_NC_CACHE = None


def _get_nc():
    global _NC_CACHE
    if _NC_CACHE is None:
        _NC_CACHE = build_bass()
    return _NC_CACHE


def kernel(x, bid_info):
    x = np.ascontiguousarray(np.asarray(x, dtype=np.float32))
    bid_info = np.ascontiguousarray(np.asarray(bid_info, dtype=np.int32))
    assert x.shape == (B, S) and bid_info.shape == (B, 2)

    nc = _get_nc()
    in_maps = [
        {
            "x": x[c * ROWS:(c + 1) * ROWS],
            "bid_info": bid_info[c * ROWS:(c + 1) * ROWS],
        }
        for c in range(N_CORES)
    ]
    res = run_bass_kernel_spmd(nc, in_maps, core_ids=list(range(N_CORES)))
    survival = np.concatenate([r["survival"] for r in res.results], axis=0)
    rate_last = np.concatenate([r["rate_last"] for r in res.results], axis=0)
    return survival, rate_last


# revision 32
# speedup vs baseline: 3.7626x; 1.0769x over previous
"""Trainium2 Bass kernel for nn_BidPrefix (segment_reduce).

Reference semantics, per row r (B=65536 rows, S=512 cols):
    cp[k]    = prod(x[r, 0:k])                  (exclusive prefix product)
    survival = cp[bid]
    rate     = cp[mp] - cp[mp+1], or EPS when mp == 0
returned as (survival [B,1] f32, rate_last [B,1] f32).

Design (v4): exact fp32 inclusive cumprod via DVE tensor_tensor_scan
(op0=mult, op1=bypass; one 512-long recurrence per row-group, written
in place over the x tile), then a per-row 3-element extraction with one
GPSIMD ap_gather per chunk:
    survival = cpi[bid-1]   (bid==0 -> 1, fixed up)
    g1       = cpi[mp-1]    (mp==0 handled by the EPS fixup)
    g2       = cpi[mp]
    rate     = mp==0 ? EPS : g1 - g2
ap_gather applies, for each 16-partition GPSIMD core, the index list
stored across its 16 partitions (slot s of partition p = flat position
q = s*16 + p%16) to ALL 16 channels; row p's own values land at
out[p, s*16 + p%16] and are pulled out with a static one-hot mask
(mult + segmented reduce).  Indices are pre-offset by glocal*512 so a
single gather covers a whole multi-group chunk.

Schedule: the three ~47us engine streams (DMA-in, DVE scans+extracts,
Pool gathers) are software-pipelined: extract for chunk c-1 is issued
after the scans of chunk c so the in-order DVE never stalls on the
Pool gather it consumes.  Chunk sizes taper (4,8...8,2,2 groups) to
shorten pipeline fill and drain; index prep runs on the (otherwise
idle until the first gather) Pool engine; fixups are split so only the
last chunk's sliver trails the final gather.

Row mapping r = p*64 + k keeps every DMA contiguous per partition:
x chunks 4-16KB, bid_info 512B, outputs 256B.

Sharding: pure data parallel over the batch axis, B/8 = 8192 rows per
NeuronCore, same NEFF on all 8 cores (SPMD), outputs concatenated.
"""

import numpy as np

import concourse.bacc as bacc
import concourse.mybir as mybir
from concourse.tile import TileContext
from concourse.bass_utils import run_bass_kernel_spmd

f32 = mybir.dt.float32
i32 = mybir.dt.int32
i16 = mybir.dt.int16
Alu = mybir.AluOpType
Ax = mybir.AxisListType

N_CORES = 8
B, S = 65536, 512
ROWS = B // N_CORES          # 8192 rows per core
K = ROWS // 128              # 64 row-groups per partition
CHUNKS = [4] * 12 + [2] * 8                # groups per chunk, sum = 64
EBATCH = 1                                 # chunks per extract batch
DEFER = 2                                  # batches between gather and extract
SPLIT_HEAD = 0                             # leading chunks with per-group DMA
assert sum(CHUNKS) == K
EPS = 1e-7


def build_bass():
    nc = bacc.Bacc()

    x = nc.dram_tensor("x", [ROWS, S], f32, kind="ExternalInput")
    bid_info = nc.dram_tensor("bid_info", [ROWS, 2], i32, kind="ExternalInput")
    out2 = nc.dram_tensor("out2", [ROWS, 2], f32, kind="ExternalOutput")

    # row r = p*64 + k  ->  every DMA contiguous per partition
    x_v = x.rearrange("(p k) s -> p (k s)", p=128)           # [128, 64*512]
    bi_v = bid_info.rearrange("(p k) c -> p (k c)", p=128)   # [128, 128]
    o2_v = out2.rearrange("(p k) c -> p (k c)", p=128)       # [128, 128]

    import contextlib

    # Ring depth per chunk-size class: deep enough to keep the DMA stream
    # gapless, within a ~120KB/partition budget split across classes.
    classes = sorted(set(CHUNKS))
    per_class_kb = 120 // len(classes)
    BUFS = {gc: max(2, min(10, per_class_kb // (2 * gc))) for gc in classes}

    with TileContext(nc) as tc:
        with contextlib.ExitStack() as stack:
            cpool = stack.enter_context(tc.tile_pool(name="const", bufs=1))
            spool = stack.enter_context(tc.tile_pool(name="small", bufs=3))
            pools = {
                gc: stack.enter_context(
                    tc.tile_pool(name=f"big{gc}", bufs=BUFS[gc]))
                for gc in classes
            }

            # ---- bid_info first (tiny; unblocks index prep), then x chunk 0
            bi_all = cpool.tile([128, 2 * K], i32, tag="bi_all")
            nc.sync.dma_start(out=bi_all[:], in_=bi_v)
            g0 = CHUNKS[0]
            xt0 = pools[g0].tile([128, g0 * S], f32, tag=f"xt{g0}")
            w0 = S if SPLIT_HEAD > 0 else g0 * S
            nc.sync.dma_start(out=xt0[:, 0:w0], in_=x_v[:, 0:w0])

            # ---- static constants + index prep, all on Pool (idle until
            # the first gather) so the DVE can start scanning immediately --
            MW = 48 * max(sum(CHUNKS[i:i + EBATCH])
                          for i in range(0, len(CHUNKS), EBATCH))
            rq = cpool.tile([128, MW], i32, tag="rq")          # q%16 per slot
            nc.gpsimd.iota(rq[:], pattern=[[0, MW // 16], [1, 16]], base=0,
                           channel_multiplier=0)
            pp = cpool.tile([128, 1], i32, tag="pp")           # partition idx
            nc.gpsimd.iota(pp[:], pattern=[[1, 1]], base=0,
                           channel_multiplier=1)
            offs32 = cpool.tile([128, 3 * K], i32, tag="offs32")  # glocal*512
            gs = 0
            for gc in CHUNKS:
                nc.gpsimd.iota(offs32[:, 3 * gs:3 * (gs + gc)],
                               pattern=[[512, gc], [0, 3]], base=0,
                               channel_multiplier=0)
                gs += gc

            # Pool handles the casts (proven gpsimd ops); the
            # TensorScalarPtr-encoded arithmetic must stay on DVE (walrus
            # rejects that encoding on Pool).
            pm16 = cpool.tile([128, 1], i32, tag="pm16")
            nc.vector.tensor_scalar(out=pm16[:], in0=pp[:], scalar1=15,
                                    scalar2=None, op0=Alu.bitwise_and)
            pm16f = cpool.tile([128, 1], f32, tag="pm16f")
            nc.gpsimd.tensor_copy(out=pm16f[:], in_=pm16[:])
            rqf = cpool.tile([128, MW], f32, tag="rqf")
            nc.gpsimd.tensor_copy(out=rqf[:], in_=rq[:])
            # one-hot diag mask M[p, s*16+r] = (r == p%16), periodic in s
            m384 = cpool.tile([128, MW], f32, tag="m384")
            nc.vector.tensor_scalar(out=m384[:], in0=rqf[:], scalar1=pm16f[:],
                                    scalar2=None, op0=Alu.is_equal)
            offsf = cpool.tile([128, 3 * K], f32, tag="offsf")
            nc.gpsimd.tensor_copy(out=offsf[:], in_=offs32[:])

            bif = cpool.tile([128, 2 * K], f32, tag="bif")
            nc.gpsimd.tensor_copy(out=bif[:], in_=bi_all[:])
            bif3 = bif[:].rearrange("p (k c) -> p k c", c=2)
            mpf = bif3[:, :, 0]     # [128, 64] market price (strided)
            bidf = bif3[:, :, 1]    # [128, 64] bid

            idxf = cpool.tile([128, 3 * K], f32, tag="idxf")
            idx3 = idxf[:].rearrange("p (k j) -> p k j", j=3)
            # j=0: bid-1, j=1: mp-1, j=2: mp   (clamped at 0; fixups later)
            nc.vector.tensor_scalar(out=idx3[:, :, 0], in0=bidf, scalar1=-1.0,
                                    scalar2=0.0, op0=Alu.add, op1=Alu.max)
            nc.vector.tensor_scalar(out=idx3[:, :, 1], in0=mpf, scalar1=-1.0,
                                    scalar2=0.0, op0=Alu.add, op1=Alu.max)
            nc.gpsimd.tensor_copy(out=idx3[:, :, 2], in_=mpf)
            nc.vector.tensor_tensor(out=idxf[:], in0=idxf[:], in1=offsf[:],
                                    op=Alu.add)
            idx16 = cpool.tile([128, 3 * K], i16, tag="idx16")
            nc.gpsimd.tensor_copy(out=idx16[:], in_=idxf[:])

            # masks + fill constants for the bid==0 / mp==0 fixups
            # (CopyPredicated wants an integer mask dtype)
            mb = cpool.tile([128, K], i32, tag="mb")
            nc.vector.tensor_scalar(out=mb[:], in0=bidf, scalar1=0.0,
                                    scalar2=None, op0=Alu.is_equal)
            mm = cpool.tile([128, K], i32, tag="mm")
            nc.vector.tensor_scalar(out=mm[:], in0=mpf, scalar1=0.0,
                                    scalar2=None, op0=Alu.is_equal)
            ones = cpool.tile([128, K], f32, tag="ones")
            nc.gpsimd.memset(ones[:], 1.0)
            epsc = cpool.tile([128, K], f32, tag="epsc")
            nc.gpsimd.memset(epsc[:], EPS)

            vals = cpool.tile([128, 3 * K], f32, tag="vals")  # [p, k, j]
            ost = cpool.tile([128, 2 * K], f32, tag="ost")    # [p, k, (s,r)]
            ost3 = ost[:].rearrange("p (k c) -> p k c", c=2)
            surv = ost3[:, :, 0]    # strided [128, 64] views
            rate = ost3[:, :, 1]

            v3 = vals[:].rearrange("p (k j) -> p k j", j=3)

            def extract(bgs, bw, gat):
                msk = spool.tile([128, 48 * bw], f32, tag=f"msk{bw}")
                nc.vector.tensor_tensor(out=msk[:], in0=gat[:],
                                        in1=m384[:, 0:48 * bw], op=Alu.mult)
                m3 = msk[:].rearrange("p (s r) -> p s r", r=16)
                nc.vector.tensor_reduce(out=vals[:, 3 * bgs:3 * (bgs + bw)],
                                        in_=m3, axis=Ax.X, op=Alu.add)

            def fix_surv(lo, hi):
                sl = slice(lo, hi)
                # survival = bid==0 ? 1 : cpi[bid-1]
                nc.vector.tensor_copy(out=surv[:, sl], in_=v3[:, sl, 0])
                nc.vector.copy_predicated(out=surv[:, sl], mask=mb[:, sl],
                                          data=ones[:, sl])

            def fix_rate(lo, hi):
                sl = slice(lo, hi)
                # rate = mp==0 ? EPS : cpi[mp-1] - cpi[mp]
                nc.vector.tensor_tensor(out=rate[:, sl], in0=v3[:, sl, 1],
                                        in1=v3[:, sl, 2], op=Alu.subtract)
                nc.vector.copy_predicated(out=rate[:, sl], mask=mm[:, sl],
                                          data=epsc[:, sl])

            def fixups(lo, hi):
                fix_surv(lo, hi)
                fix_rate(lo, hi)

            # ---- main loop over chunks -----------------------------------
            # Gathers land in a shared per-batch tile; one mult+reduce
            # extracts a whole batch (fewer DVE instructions).  Extraction
            # of batch b is issued once batch b+1 is complete, so the
            # in-order DVE meets long-finished Pool gathers.
            batch_w = [sum(CHUNKS[i:i + EBATCH])
                       for i in range(0, len(CHUNKS), EBATCH)]
            pending = []    # (bgs, bw, gat) full batches awaiting extraction
            cur = None      # [bgs, bw, gat, filled] batch being filled
            gs = 0
            n_chunks = len(CHUNKS)
            for ci, gc in enumerate(CHUNKS):
                # The first chunks' DMAs are split per group so their scans
                # (and so the first gathers) chase the DMA stream instead of
                # waiting for the whole chunk to land.
                split = ci < SPLIT_HEAD
                if ci == 0:
                    xt = xt0
                else:
                    xt = pools[gc].tile([128, gc * S], f32, tag=f"xt{gc}")
                    if not split:
                        nc.sync.dma_start(out=xt[:],
                                          in_=x_v[:, gs * S:(gs + gc) * S])
                if split:
                    for g in range(gc):
                        if ci == 0 and g == 0:
                            continue  # xt0's first slice DMA issued up top
                        nc.sync.dma_start(
                            out=xt[:, g * S:(g + 1) * S],
                            in_=x_v[:, (gs + g) * S:(gs + g + 1) * S])

                for g in range(gc):
                    sl = slice(g * S, (g + 1) * S)
                    if split or g == 0:
                        # Tiny read absorbs the HWDGE queue semaphore before
                        # the TensorScalarPtr-encoded scans (that ISA encoding
                        # has too few sync-wait slots to carry it itself).
                        sink = spool.tile([128, 2], f32, tag="sink")
                        nc.vector.tensor_copy(out=sink[:, 0:1],
                                              in_=xt[:, sl.start:sl.start + 1])
                    nc.vector.tensor_tensor_scan(
                        out=xt[:, sl], data0=xt[:, sl], data1=xt[:, sl],
                        initial=1.0, op0=Alu.mult, op1=Alu.bypass)

                if cur is None:
                    bw = batch_w[ci // EBATCH]
                    gat = spool.tile([128, 48 * bw], f32, tag=f"gatb{bw}")
                    cur = [gs, bw, gat, 0]
                off = cur[3]
                nc.gpsimd.ap_gather(
                    out_ap=cur[2][:, 48 * off:48 * (off + gc)], in_ap=xt[:],
                    idxs_ap=idx16[:, 3 * gs:3 * (gs + gc)],
                    channels=128, num_elems=gc * S, d=1, num_idxs=48 * gc)
                cur[3] += gc

                if cur[3] == cur[1]:
                    pending.append((cur[0], cur[1], cur[2]))
                    cur = None
                    while len(pending) > DEFER:
                        extract(*pending.pop(0))
                if ci == n_chunks - 1:
                    # head fixups run on DVE while the last gathers run
                    while len(pending) > 1:
                        extract(*pending.pop(0))
                    fixups(0, pending[0][0])
                gs += gc

            tail_lo = pending[0][0]
            extract(*pending.pop(0))
            fix_surv(tail_lo, K)
            fix_rate(tail_lo, K)
            nc.sync.dma_start(out=o2_v, in_=ost[:])
    nc.finalize()
    return nc


_NC_CACHE = None


def _get_nc():
    global _NC_CACHE
    if _NC_CACHE is None:
        _NC_CACHE = build_bass()
    return _NC_CACHE


def kernel(x, bid_info):
    x = np.ascontiguousarray(np.asarray(x, dtype=np.float32))
    bid_info = np.ascontiguousarray(np.asarray(bid_info, dtype=np.int32))
    assert x.shape == (B, S) and bid_info.shape == (B, 2)

    nc = _get_nc()
    in_maps = [
        {
            "x": x[c * ROWS:(c + 1) * ROWS],
            "bid_info": bid_info[c * ROWS:(c + 1) * ROWS],
        }
        for c in range(N_CORES)
    ]
    res = run_bass_kernel_spmd(nc, in_maps, core_ids=list(range(N_CORES)))
    out2 = np.concatenate([r["out2"] for r in res.results], axis=0)
    return np.ascontiguousarray(out2[:, 0:1]), np.ascontiguousarray(out2[:, 1:2])


# revision 37
# speedup vs baseline: 3.7709x; 1.0022x over previous
"""Trainium2 Bass kernel for nn_BidPrefix (segment_reduce).

Reference semantics, per row r (B=65536 rows, S=512 cols):
    cp[k]    = prod(x[r, 0:k])                  (exclusive prefix product)
    survival = cp[bid]
    rate     = cp[mp] - cp[mp+1], or EPS when mp == 0
returned as (survival [B,1] f32, rate_last [B,1] f32).

Design: exact fp32 inclusive cumprod via DVE tensor_tensor_scan
(op0=mult, op1=bypass; one 512-long recurrence per row-group, written
in place over the x tile), then a per-row 3-element extraction with one
GPSIMD ap_gather per chunk:
    survival = cpi[bid-1]   (bid==0 -> 1, fixed up)
    g1       = cpi[mp-1]    (mp==0 handled by the EPS fixup)
    g2       = cpi[mp]
    rate     = mp==0 ? EPS : g1 - g2
ap_gather applies, for each 16-partition GPSIMD core, the index list
stored across its 16 partitions (slot s of partition p = flat position
q = s*16 + p%16) to ALL 16 channels; row p's own values land at
out[p, s*16 + p%16] and are pulled out with a static one-hot mask
(mult + segmented reduce).  Indices are pre-offset by glocal*512 so a
single gather covers a whole multi-group chunk.

Schedule: the three ~47us engine streams (DMA-in, DVE scans+extracts,
Pool gathers) are software-pipelined: a chunk's extract is deferred
DEFER chunks so the in-order DVE never stalls on the Pool gather it
consumes.  Chunk sizes taper (3,3,4..4,2..2 groups) to shorten
pipeline fill and drain; index prep casts run on the (otherwise idle
until the first gather) Pool engine; fixups are predicated copies,
split so only the last chunk's sliver trails the final gather; both
outputs interleave into one DRAM tensor so the tail pays a single
DMA chain.

Row mapping r = p*64 + k keeps every DMA contiguous per partition:
x chunks 4-16KB, bid_info 512B, output 512B.

Sharding: pure data parallel over the batch axis, B/8 = 8192 rows per
NeuronCore, same NEFF on all 8 cores (SPMD), outputs concatenated.
"""

import numpy as np

import concourse.bacc as bacc
import concourse.mybir as mybir
from concourse.tile import TileContext
from concourse.bass_utils import run_bass_kernel_spmd

f32 = mybir.dt.float32
i32 = mybir.dt.int32
i16 = mybir.dt.int16
Alu = mybir.AluOpType
Ax = mybir.AxisListType

N_CORES = 8
B, S = 65536, 512
ROWS = B // N_CORES          # 8192 rows per core
K = ROWS // 128              # 64 row-groups per partition
# Chunk sizes must be EVEN: a chunk's int16 index slice is 3*gc slots per
# partition, and the hardware ap_gather ucode needs 4-byte-aligned index
# slices (odd gc -> 18-byte offsets -> silently wrong gathers on HW, even
# though the interpreter/cost model accept it).
CHUNKS = [4] * 11 + [2] * 10                # groups per chunk, sum = 64
EBATCH = 1                                 # chunks per extract batch
DEFER = 2                                  # batches between gather and extract
SPLIT_HEAD = 0                             # leading chunks with per-group DMA
assert sum(CHUNKS) == K
assert all(gc % 2 == 0 for gc in CHUNKS), "odd chunks break ap_gather on HW"
EPS = 1e-7


def build_bass():
    nc = bacc.Bacc()

    x = nc.dram_tensor("x", [ROWS, S], f32, kind="ExternalInput")
    bid_info = nc.dram_tensor("bid_info", [ROWS, 2], i32, kind="ExternalInput")
    out2 = nc.dram_tensor("out2", [ROWS, 2], f32, kind="ExternalOutput")

    # row r = p*64 + k  ->  every DMA contiguous per partition
    x_v = x.rearrange("(p k) s -> p (k s)", p=128)           # [128, 64*512]
    bi_v = bid_info.rearrange("(p k) c -> p (k c)", p=128)   # [128, 128]
    o2_v = out2.rearrange("(p k) c -> p (k c)", p=128)       # [128, 128]

    import contextlib

    # Ring depth per chunk-size class: deep enough to keep the DMA stream
    # gapless, within a ~120KB/partition budget split across classes.
    classes = sorted(set(CHUNKS))
    per_class_kb = 120 // len(classes)
    BUFS = {gc: max(2, min(10, per_class_kb // (2 * gc))) for gc in classes}

    with TileContext(nc) as tc:
        with contextlib.ExitStack() as stack:
            cpool = stack.enter_context(tc.tile_pool(name="const", bufs=1))
            spool = stack.enter_context(tc.tile_pool(name="small", bufs=5))
            pools = {
                gc: stack.enter_context(
                    tc.tile_pool(name=f"big{gc}", bufs=BUFS[gc]))
                for gc in classes
            }

            # ---- bid_info first (tiny; unblocks index prep), then x chunk 0
            bi_all = cpool.tile([128, 2 * K], i32, tag="bi_all")
            nc.sync.dma_start(out=bi_all[:], in_=bi_v)
            g0 = CHUNKS[0]
            xt0 = pools[g0].tile([128, g0 * S], f32, tag=f"xt{g0}")
            w0 = S if SPLIT_HEAD > 0 else g0 * S
            nc.sync.dma_start(out=xt0[:, 0:w0], in_=x_v[:, 0:w0])

            # ---- static constants + index prep, all on Pool (idle until
            # the first gather) so the DVE can start scanning immediately --
            MW = 48 * max(sum(CHUNKS[i:i + EBATCH])
                          for i in range(0, len(CHUNKS), EBATCH))
            rq = cpool.tile([128, MW], i32, tag="rq")          # q%16 per slot
            nc.gpsimd.iota(rq[:], pattern=[[0, MW // 16], [1, 16]], base=0,
                           channel_multiplier=0)
            pp = cpool.tile([128, 1], i32, tag="pp")           # partition idx
            nc.gpsimd.iota(pp[:], pattern=[[1, 1]], base=0,
                           channel_multiplier=1)
            offs32 = cpool.tile([128, 3 * K], i32, tag="offs32")  # glocal*512
            gs = 0
            for gc in CHUNKS:
                nc.gpsimd.iota(offs32[:, 3 * gs:3 * (gs + gc)],
                               pattern=[[512, gc], [0, 3]], base=0,
                               channel_multiplier=0)
                gs += gc

            # Pool handles the casts (proven gpsimd ops); the
            # TensorScalarPtr-encoded arithmetic must stay on DVE (walrus
            # rejects that encoding on Pool).
            pm16 = cpool.tile([128, 1], i32, tag="pm16")
            nc.vector.tensor_scalar(out=pm16[:], in0=pp[:], scalar1=15,
                                    scalar2=None, op0=Alu.bitwise_and)
            pm16f = cpool.tile([128, 1], f32, tag="pm16f")
            nc.gpsimd.tensor_copy(out=pm16f[:], in_=pm16[:])
            rqf = cpool.tile([128, MW], f32, tag="rqf")
            nc.gpsimd.tensor_copy(out=rqf[:], in_=rq[:])
            # one-hot diag mask M[p, s*16+r] = (r == p%16), periodic in s
            m384 = cpool.tile([128, MW], f32, tag="m384")
            nc.vector.tensor_scalar(out=m384[:], in0=rqf[:], scalar1=pm16f[:],
                                    scalar2=None, op0=Alu.is_equal)
            offsf = cpool.tile([128, 3 * K], f32, tag="offsf")
            nc.gpsimd.tensor_copy(out=offsf[:], in_=offs32[:])

            bif = cpool.tile([128, 2 * K], f32, tag="bif")
            nc.gpsimd.tensor_copy(out=bif[:], in_=bi_all[:])
            bif3 = bif[:].rearrange("p (k c) -> p k c", c=2)
            mpf = bif3[:, :, 0]     # [128, 64] market price (strided)
            bidf = bif3[:, :, 1]    # [128, 64] bid

            idxf = cpool.tile([128, 3 * K], f32, tag="idxf")
            idx3 = idxf[:].rearrange("p (k j) -> p k j", j=3)
            # j=0: bid-1, j=1: mp-1, j=2: mp   (clamped at 0; fixups later)
            nc.vector.tensor_scalar(out=idx3[:, :, 0], in0=bidf, scalar1=-1.0,
                                    scalar2=0.0, op0=Alu.add, op1=Alu.max)
            nc.vector.tensor_scalar(out=idx3[:, :, 1], in0=mpf, scalar1=-1.0,
                                    scalar2=0.0, op0=Alu.add, op1=Alu.max)
            nc.gpsimd.tensor_copy(out=idx3[:, :, 2], in_=mpf)
            nc.vector.tensor_tensor(out=idxf[:], in0=idxf[:], in1=offsf[:],
                                    op=Alu.add)
            idx16 = cpool.tile([128, 3 * K], i16, tag="idx16")
            nc.gpsimd.tensor_copy(out=idx16[:], in_=idxf[:])

            # masks + fill constants for the bid==0 / mp==0 fixups
            # (CopyPredicated wants an integer mask dtype)
            mb = cpool.tile([128, K], i32, tag="mb")
            nc.vector.tensor_scalar(out=mb[:], in0=bidf, scalar1=0.0,
                                    scalar2=None, op0=Alu.is_equal)
            mm = cpool.tile([128, K], i32, tag="mm")
            nc.vector.tensor_scalar(out=mm[:], in0=mpf, scalar1=0.0,
                                    scalar2=None, op0=Alu.is_equal)
            ones = cpool.tile([128, K], f32, tag="ones")
            nc.gpsimd.memset(ones[:], 1.0)
            epsc = cpool.tile([128, K], f32, tag="epsc")
            nc.gpsimd.memset(epsc[:], EPS)

            vals = cpool.tile([128, 3 * K], f32, tag="vals")  # [p, k, j]
            ost = cpool.tile([128, 2 * K], f32, tag="ost")    # [p, k, (s,r)]
            ost3 = ost[:].rearrange("p (k c) -> p k c", c=2)
            surv = ost3[:, :, 0]    # strided [128, 64] views
            rate = ost3[:, :, 1]

            v3 = vals[:].rearrange("p (k j) -> p k j", j=3)

            def extract(bgs, bw, gat):
                msk = spool.tile([128, 48 * bw], f32, tag=f"msk{bw}")
                nc.vector.tensor_tensor(out=msk[:], in0=gat[:],
                                        in1=m384[:, 0:48 * bw], op=Alu.mult)
                m3 = msk[:].rearrange("p (s r) -> p s r", r=16)
                nc.vector.tensor_reduce(out=vals[:, 3 * bgs:3 * (bgs + bw)],
                                        in_=m3, axis=Ax.X, op=Alu.add)

            def fix_surv(lo, hi):
                sl = slice(lo, hi)
                # survival = bid==0 ? 1 : cpi[bid-1]
                nc.vector.tensor_copy(out=surv[:, sl], in_=v3[:, sl, 0])
                nc.vector.copy_predicated(out=surv[:, sl], mask=mb[:, sl],
                                          data=ones[:, sl])

            def fix_rate(lo, hi):
                sl = slice(lo, hi)
                # rate = mp==0 ? EPS : cpi[mp-1] - cpi[mp]
                nc.vector.tensor_tensor(out=rate[:, sl], in0=v3[:, sl, 1],
                                        in1=v3[:, sl, 2], op=Alu.subtract)
                nc.vector.copy_predicated(out=rate[:, sl], mask=mm[:, sl],
                                          data=epsc[:, sl])

            def fixups(lo, hi):
                fix_surv(lo, hi)
                fix_rate(lo, hi)

            # ---- main loop over chunks -----------------------------------
            # Gathers land in a shared per-batch tile; one mult+reduce
            # extracts a whole batch (fewer DVE instructions).  Extraction
            # of batch b is issued once batch b+1 is complete, so the
            # in-order DVE meets long-finished Pool gathers.
            batch_w = [sum(CHUNKS[i:i + EBATCH])
                       for i in range(0, len(CHUNKS), EBATCH)]
            pending = []    # (bgs, bw, gat) full batches awaiting extraction
            cur = None      # [bgs, bw, gat, filled] batch being filled
            gs = 0
            n_chunks = len(CHUNKS)
            for ci, gc in enumerate(CHUNKS):
                # The first chunks' DMAs are split per group so their scans
                # (and so the first gathers) chase the DMA stream instead of
                # waiting for the whole chunk to land.
                split = ci < SPLIT_HEAD
                if ci == 0:
                    xt = xt0
                else:
                    xt = pools[gc].tile([128, gc * S], f32, tag=f"xt{gc}")
                    if not split:
                        nc.sync.dma_start(out=xt[:],
                                          in_=x_v[:, gs * S:(gs + gc) * S])
                if split:
                    for g in range(gc):
                        if ci == 0 and g == 0:
                            continue  # xt0's first slice DMA issued up top
                        nc.sync.dma_start(
                            out=xt[:, g * S:(g + 1) * S],
                            in_=x_v[:, (gs + g) * S:(gs + g + 1) * S])

                for g in range(gc):
                    sl = slice(g * S, (g + 1) * S)
                    if split or g == 0:
                        # Tiny read absorbs the HWDGE queue semaphore before
                        # the TensorScalarPtr-encoded scans (that ISA encoding
                        # has too few sync-wait slots to carry it itself).
                        sink = spool.tile([128, 2], f32, tag="sink")
                        nc.vector.tensor_copy(out=sink[:, 0:1],
                                              in_=xt[:, sl.start:sl.start + 1])
                    nc.vector.tensor_tensor_scan(
                        out=xt[:, sl], data0=xt[:, sl], data1=xt[:, sl],
                        initial=1.0, op0=Alu.mult, op1=Alu.bypass)

                if cur is None:
                    bw = batch_w[ci // EBATCH]
                    gat = spool.tile([128, 48 * bw], f32, tag=f"gatb{bw}")
                    cur = [gs, bw, gat, 0]
                off = cur[3]
                nc.gpsimd.ap_gather(
                    out_ap=cur[2][:, 48 * off:48 * (off + gc)], in_ap=xt[:],
                    idxs_ap=idx16[:, 3 * gs:3 * (gs + gc)],
                    channels=128, num_elems=gc * S, d=1, num_idxs=48 * gc)
                cur[3] += gc

                if cur[3] == cur[1]:
                    pending.append((cur[0], cur[1], cur[2]))
                    cur = None
                    while len(pending) > DEFER:
                        extract(*pending.pop(0))
                if ci == n_chunks - 1:
                    # head fixups run on DVE while the last gathers run
                    while len(pending) > 1:
                        extract(*pending.pop(0))
                    fixups(0, pending[0][0])
                gs += gc

            tail_lo = pending[0][0]
            extract(*pending.pop(0))
            fix_surv(tail_lo, K)
            fix_rate(tail_lo, K)
            nc.sync.dma_start(out=o2_v, in_=ost[:])
    nc.finalize()
    return nc


_NC_CACHE = None


def _get_nc():
    global _NC_CACHE
    if _NC_CACHE is None:
        _NC_CACHE = build_bass()
    return _NC_CACHE


def kernel(x, bid_info):
    x = np.ascontiguousarray(np.asarray(x, dtype=np.float32))
    bid_info = np.ascontiguousarray(np.asarray(bid_info, dtype=np.int32))
    assert x.shape == (B, S) and bid_info.shape == (B, 2)

    nc = _get_nc()
    in_maps = [
        {
            "x": x[c * ROWS:(c + 1) * ROWS],
            "bid_info": bid_info[c * ROWS:(c + 1) * ROWS],
        }
        for c in range(N_CORES)
    ]
    res = run_bass_kernel_spmd(nc, in_maps, core_ids=list(range(N_CORES)))
    out2 = np.concatenate([r["out2"] for r in res.results], axis=0)
    return np.ascontiguousarray(out2[:, 0:1]), np.ascontiguousarray(out2[:, 1:2])
